# revision 1
# baseline (speedup 1.0000x reference)
"""KAN-SE (squeeze-excite with 2-layer KAN MLP) Trainium2 kernel.

Full-input contract: kernel(**inputs) takes the complete (32, 512, 64, 64)
batch plus KAN weights, shards the batch across 8 NeuronCores (4 samples
per core, data-parallel, weights replicated), and returns the full output.

Per-core device program (pure SPMD, no collectives):
  for each of 4 samples:
    - load the sample's (512, 4096) pixels as 4 tiles of (128, 4096) f32,
      keep them resident in SBUF
    - per-channel mean via free-dim reduce  -> s (512,)
    - 2-layer KAN on s (B-spline bases via Cox-de-Boor on VectorE,
      einsums as tiny PE matmuls accumulating in PSUM, SiLU/Sigmoid on
      ScalarE) -> per-channel gate (512,)
    - scale the resident tiles by the gate and store

x is read exactly once (SBUF-resident between mean and scale), so HBM
traffic is the 2x minimum: 8 MiB in + 8 MiB out per sample per core.
"""

import numpy as np

# ---- problem constants (hardcoded per contract; do not read spec/reference) ----
B, C, H, W = 32, 512, 64, 64
HIDDEN = 64            # max(16, 512 // 8)
KB = 8                 # GRID_SIZE + SPLINE_ORDER = 5 + 3
NCORES = 8
NS = B // NCORES       # samples per core = 4
NG = C // 128          # channel groups of 128 = 4
HWPIX = H * W          # 4096

# gtab column layout: [G0(12) | -g_i for k=1(10) | g_{i+2} k=1(10)
#                      | -g_i k=2(9) | g_{i+3} k=2(9) | -g_i k=3(8) | g_{i+4} k=3(8)]
_GT_OFF = {"G0": 0, 1: (12, 22), 2: (32, 41), 3: (50, 58)}
_GT_W = 66


def _grid_tables(grid_row: np.ndarray):
    """Build the (128, 66) constant table + per-level reciprocal immediates
    from one row of the (uniform) grid."""
    g = np.asarray(grid_row, np.float64)
    assert g.shape == (12,)
    h = g[1] - g[0]
    tab = np.zeros((_GT_W,), np.float64)
    tab[0:12] = g
    rs = {}
    for k in (1, 2, 3):
        w = 11 - k
        aoff, coff = _GT_OFF[k]
        tab[aoff:aoff + w] = -g[:w]          # -g_i,      i = 0..10-k
        tab[coff:coff + w] = g[k + 1:12]     # g_{i+k+1}, i = 0..10-k
        rs[k] = float(np.float32(1.0 / (k * h)))
    full = np.tile(tab.astype(np.float32)[None, :], (128, 1))
    return np.ascontiguousarray(full), rs


def _host_prep(inputs):
    """Rearrange weights into the SBUF layouts the device program uses."""
    f32 = np.float32
    base_w1 = np.asarray(inputs["base_w1"], f32)      # (64, 512)
    spline_w1 = np.asarray(inputs["spline_w1"], f32)  # (64, 512, 8)
    scaler1 = np.asarray(inputs["scaler1"], f32)      # (64, 512)
    base_w2 = np.asarray(inputs["base_w2"], f32)      # (512, 64)
    spline_w2 = np.asarray(inputs["spline_w2"], f32)  # (512, 64, 8)
    scaler2 = np.asarray(inputs["scaler2"], f32)      # (512, 64)

    # w1t[p, g*64+o] = base_w1[o, 128g+p]
    w1t = base_w1.reshape(HIDDEN, NG, 128).transpose(2, 1, 0).reshape(128, NG * HIDDEN)
    # sw1[p, (g*8+k)*64+o] = (spline_w1*scaler1)[o, 128g+p, k]
    sw1 = (spline_w1 * scaler1[:, :, None]).reshape(HIDDEN, NG, 128, KB)
    sw1 = sw1.transpose(2, 1, 3, 0).reshape(128, NG * KB * HIDDEN)
    # w2t[p, o] = base_w2[o, p]
    w2t = base_w2.T
    # sw2[p, k*512+o] = (spline_w2*scaler2)[o, p, k]
    sw2 = (spline_w2 * scaler2[:, :, None]).transpose(1, 2, 0).reshape(HIDDEN, KB * C)

    gt1, rs1 = _grid_tables(np.asarray(inputs["grid1"], f32)[0])
    gt2, rs2 = _grid_tables(np.asarray(inputs["grid2"], f32)[0])

    tensors = {
        "w1t": np.ascontiguousarray(w1t, f32),
        "sw1": np.ascontiguousarray(sw1, f32),
        "w2t": np.ascontiguousarray(w2t, f32),
        "sw2": np.ascontiguousarray(sw2, f32),
        "gt1": gt1,
        "gt2": gt2,
    }
    return tensors, rs1, rs2


def _emit_bsplines(nc, mybir, pool, gt_sb, x_ap, out_ap, p, rs):
    """Cubic B-spline bases of x (one value per partition) -> out_ap (p, 8).

    Cox-de-Boor on VectorE with per-basis-index grid constants from gt_sb
    and uniform-knot reciprocals rs (immediates).
    """
    f32 = mybir.dt.float32
    Alu = mybir.AluOpType
    ge = pool.tile([128, 12], f32, tag="ge", bufs=4)
    # ge[:, i] = (g_i <= x)
    nc.vector.tensor_scalar(
        out=ge[:p], in0=gt_sb[:p, 0:12], scalar1=x_ap, scalar2=None, op0=Alu.is_le
    )
    bprev = pool.tile([128, 11], f32, tag="b0", bufs=4)
    nc.vector.tensor_tensor(bprev[:p], ge[:p, 0:11], ge[:p, 1:12], Alu.subtract)
    for k in (1, 2, 3):
        w = 11 - k
        aoff, coff = _GT_OFF[k]
        a_t = pool.tile([128, 10], f32, tag="bsA", bufs=4)
        c_t = pool.tile([128, 10], f32, tag="bsC", bufs=4)
        # A = (x - g_i) / (k h);  C = (g_{i+k+1} - x) / (k h)
        nc.vector.tensor_scalar(
            out=a_t[:p, :w], in0=gt_sb[:p, aoff:aoff + w], scalar1=x_ap,
            scalar2=rs[k], op0=Alu.add, op1=Alu.mult,
        )
        nc.vector.tensor_scalar(
            out=c_t[:p, :w], in0=gt_sb[:p, coff:coff + w], scalar1=x_ap,
            scalar2=rs[k], op0=Alu.subtract, op1=Alu.mult,
        )
        if k < 3:
            bnext = pool.tile([128, 10], f32, tag="bn", bufs=4)
            outp = bnext[:p, :w]
        else:
            outp = out_ap
        nc.vector.tensor_tensor(c_t[:p, :w], c_t[:p, :w], bprev[:p, 1:w + 1], Alu.mult)
        nc.vector.tensor_tensor(outp, a_t[:p, :w], bprev[:p, 0:w], Alu.mult)
        nc.vector.tensor_tensor(outp, outp, c_t[:p, :w], Alu.add)
        if k < 3:
            bprev = bnext


def _build_nc(rs1, rs2):
    import concourse.bacc as bacc
    import concourse.bass as bass  # noqa: F401
    import concourse.mybir as mybir
    from concourse.tile import TileContext

    f32 = mybir.dt.float32
    Alu = mybir.AluOpType
    Act = mybir.ActivationFunctionType
    AX = mybir.AxisListType

    # Bacc (not plain Bass): its compile() runs move_matmul_waits_to_ldweights
    # + generate_event_semaphores, which split multi-waits down to the 1-wait-
    # per-instruction TRN2 ISA limit that walrus enforces.
    nc = bacc.Bacc("TRN2", target_bir_lowering=False)
    x_d = nc.declare_dram_parameter("x", [NS, C, H, W], f32, isOutput=False)
    w1t_d = nc.declare_dram_parameter("w1t", [128, NG * HIDDEN], f32, isOutput=False)
    sw1_d = nc.declare_dram_parameter("sw1", [128, NG * KB * HIDDEN], f32, isOutput=False)
    w2t_d = nc.declare_dram_parameter("w2t", [HIDDEN, C], f32, isOutput=False)
    sw2_d = nc.declare_dram_parameter("sw2", [HIDDEN, KB * C], f32, isOutput=False)
    gt1_d = nc.declare_dram_parameter("gt1", [128, _GT_W], f32, isOutput=False)
    gt2_d = nc.declare_dram_parameter("gt2", [128, _GT_W], f32, isOutput=False)
    y_d = nc.declare_dram_parameter("y", [NS, C, H, W], f32, isOutput=True)

    with TileContext(nc) as tc:
        with (
            tc.tile_pool(name="consts", bufs=1) as cpool,
            tc.tile_pool(name="xdata", bufs=2 * NG) as xpool,
            tc.tile_pool(name="small", bufs=3) as spool,
            tc.tile_pool(name="bspl", bufs=1) as bpool,
            tc.tile_pool(name="psum", bufs=2, space="PSUM") as ppool,
        ):
            w1t_sb = cpool.tile([128, NG * HIDDEN], f32)
            nc.sync.dma_start(w1t_sb[:], w1t_d[:, :])
            sw1_sb = cpool.tile([128, NG * KB * HIDDEN], f32)
            nc.sync.dma_start(sw1_sb[:], sw1_d[:, :])
            w2t_sb = cpool.tile([HIDDEN, C], f32)
            nc.sync.dma_start(w2t_sb[:], w2t_d[:, :])
            sw2_sb = cpool.tile([HIDDEN, KB * C], f32)
            nc.sync.dma_start(sw2_sb[:], sw2_d[:, :])
            gt1_sb = cpool.tile([128, _GT_W], f32)
            nc.sync.dma_start(gt1_sb[:], gt1_d[:, :])
            gt2_sb = cpool.tile([128, _GT_W], f32)
            nc.sync.dma_start(gt2_sb[:], gt2_d[:, :])

            # Pre-touch every const tile on VectorE: the DMA-completion wait
            # lands on these throwaway copies, so later DVE consumers (notably
            # TensorScalarPtr ops, whose ISA format has a single wait slot)
            # never need a DMA wait of their own.
            touch = cpool.tile([128, 8], f32)
            for i, ct in enumerate((w1t_sb, sw1_sb, gt1_sb, gt2_sb)):
                nc.vector.tensor_copy(touch[:, i:i + 1], ct[:, 0:1])
            for i, ct in enumerate((w2t_sb, sw2_sb)):
                nc.vector.tensor_copy(touch[:HIDDEN, 4 + i:5 + i], ct[:, 0:1])
            # Same for TensorE: the LDWEIGHTS sub-instruction also has a single
            # wait slot, so absorb each weight tile's DMA wait into a throwaway
            # 1-column matmul before the real accumulation chains.
            pt_ps = ppool.tile([1, 4], f32, tag="pt")
            for i, ct in enumerate((w1t_sb, sw1_sb)):
                nc.tensor.matmul(pt_ps[0:1, i:i + 1], ct[:, 0:1], ct[:, 0:1],
                                 start=True, stop=True)
            for i, ct in enumerate((w2t_sb, sw2_sb)):
                nc.tensor.matmul(pt_ps[0:1, 2 + i:3 + i], ct[:HIDDEN, 0:1],
                                 ct[:HIDDEN, 0:1], start=True, stop=True)

            for n in range(NS):
                # ---- load sample, per-channel sums ----
                sT = spool.tile([128, NG], f32, tag="sT")
                xts = []
                for g in range(NG):
                    xt = xpool.tile([128, HWPIX], f32, tag="xt")
                    src = x_d[n, 128 * g:128 * (g + 1)].rearrange("p h w -> p (h w)")
                    nc.sync.dma_start(xt[:], src)
                    nc.vector.reduce_sum(sT[:, g:g + 1], xt[:], axis=AX.X)
                    xts.append(xt)
                # raw sums -> means
                nc.vector.tensor_scalar(
                    out=sT[:], in0=sT[:], scalar1=1.0 / HWPIX, scalar2=None,
                    op0=Alu.mult,
                )

                # ---- KAN layer 1: s (512,) -> h1 (64,) ----
                silu1 = spool.tile([128, NG], f32, tag="silu1")
                nc.scalar.activation(silu1[:], sT[:], Act.Silu)
                bf = spool.tile([128, NG * KB], f32, tag="bf")
                for g in range(NG):
                    _emit_bsplines(
                        nc, mybir, bpool, gt1_sb, sT[:, g:g + 1],
                        bf[:, KB * g:KB * (g + 1)], 128, rs1,
                    )
                ps1 = ppool.tile([HIDDEN, 1], f32, tag="ps1")
                mms = []
                for g in range(NG):
                    mms.append((w1t_sb[:, HIDDEN * g:HIDDEN * (g + 1)], silu1[:, g:g + 1]))
                for g in range(NG):
                    for k in range(KB):
                        col = HIDDEN * (KB * g + k)
                        mms.append((sw1_sb[:, col:col + HIDDEN], bf[:, KB * g + k:KB * g + k + 1]))
                for i, (lhsT, rhs) in enumerate(mms):
                    nc.tensor.matmul(
                        ps1[:], lhsT, rhs, start=(i == 0), stop=(i == len(mms) - 1)
                    )

                # ---- inter-layer SiLU, KAN layer 2: t (64,) -> (512,) ----
                t1 = spool.tile([HIDDEN, 1], f32, tag="t1")
                nc.scalar.activation(t1[:], ps1[:], Act.Silu)
                silu2 = spool.tile([HIDDEN, 1], f32, tag="silu2")
                nc.scalar.activation(silu2[:], t1[:], Act.Silu)
                b2f = spool.tile([HIDDEN, KB], f32, tag="b2f")
                _emit_bsplines(nc, mybir, bpool, gt2_sb, t1[:, 0:1], b2f[:], HIDDEN, rs2)

                ps2 = ppool.tile([128, NG], f32, tag="ps2")
                for og in range(NG):
                    mms2 = [(w2t_sb[:, 128 * og:128 * (og + 1)], silu2[:, 0:1])]
                    for k in range(KB):
                        col = C * k + 128 * og
                        mms2.append((sw2_sb[:, col:col + 128], b2f[:, k:k + 1]))
                    for i, (lhsT, rhs) in enumerate(mms2):
                        nc.tensor.matmul(
                            ps2[:, og:og + 1], lhsT, rhs,
                            start=(i == 0), stop=(i == len(mms2) - 1),
                        )

                gate = spool.tile([128, NG], f32, tag="gate")
                nc.scalar.activation(gate[:], ps2[:], Act.Sigmoid)

                # ---- scale resident tiles by the gate, store ----
                for g in range(NG):
                    nc.vector.tensor_scalar(
                        out=xts[g][:], in0=xts[g][:], scalar1=gate[:, g:g + 1],
                        scalar2=None, op0=Alu.mult,
                    )
                    dst = y_d[n, 128 * g:128 * (g + 1)].rearrange("p h w -> p (h w)")
                    nc.sync.dma_start(dst, xts[g][:])
    nc.compile()
    return nc


def _run(inputs, trace=False):
    from concourse.bass_utils import run_bass_kernel_spmd

    x = np.ascontiguousarray(np.asarray(inputs["x"], np.float32))
    assert x.shape == (B, C, H, W), x.shape
    tensors, rs1, rs2 = _host_prep(inputs)
    nc = _build_nc(rs1, rs2)
    in_maps = []
    for c in range(NCORES):
        m = {"x": np.ascontiguousarray(x[NS * c:NS * (c + 1)])}
        m.update(tensors)
        in_maps.append(m)
    res = run_bass_kernel_spmd(
        nc, in_maps, core_ids=list(range(NCORES)), trace=trace
    )
    out = np.concatenate([res.results[c]["y"] for c in range(NCORES)], axis=0)
    return out, res


def kernel(**inputs) -> np.ndarray:
    return _run(inputs)[0]

